# revision 40
# baseline (speedup 1.0000x reference)
"""Trainium2 Bass kernel for nn_Block_en_49469433315543 (involution block).

Computation (see reference):
  z = softplus(involution(x))          involution: per-pixel 3x3 dynamic kernel
  y = softplus(conv2d_3x3(z) + b_conv2)
with the per-pixel kernel = w_span @ relu(BN(w_reduce @ x)) + b_span.

Sharding: data-parallel over batch, one sample per NeuronCore (8 cores).
BN uses per-device batch statistics (sanctioned by the sharding spec);
measured deviation ~4e-3 vs the 2e-2 gate.  No collectives.

Schedule (v2, from trace analysis of the 219us baseline):
  - Involution MAC: DVE computes only the 9 per-tap products tmp_k =
    kern_k (x) x_shift_k; the TENSOR engine accumulates them into PSUM via
    identity matmuls (psum += I^T @ tmp_k, exact in fp32).  This halves the
    DVE-bound MAC phase (measured DVE 2-stream fp16 rate ~0.6 ns/elem; the
    8 add passes move to the otherwise-idle PE).
  - kern = wspanA^T @ [rn;1] computed c-major in 8 h-chunks (one stationary
    weight, 4x512-col matmuls per chunk) -> DVE drain -> DRAM bounce into
    [H, 9, W] h-major (144 x 256B descriptors per chunk).
  - Front pipeline: x_cm lands first; r-matmul chunks chase the load; BN
    stats at ~10us; relu/kern/drain/bounce in 8 pipelined h-chunks.
  - conv2 in 5 matmul passes per row group (vertical tap pairs (kh=0,kh=2)
    contract 128-deep via the stacked T1 = [zpad; zpad+260] tile; center
    row pair via T2 = [zpad+130; zpad+132]; one 64-deep single).
  - z transpose bounces via DRAM per (wave, row-group); T1-lower/T2 are
    flat shifted SBUF->SBUF copies of the landed zpad.
  - GpSimd never computes (measured 4x DVE slowdown from SBUF contention);
    it only kicks software-DGE DMAs.
"""
import sys

for _p in ("/opt/trn_rl_repo", "/root/.axon_site/_ro/trn_rl_repo"):
    if _p not in sys.path:
        sys.path.insert(0, _p)

import numpy as np

import concourse.bacc as bacc
import concourse.tile as tile
from concourse import mybir
from concourse.bass_utils import run_bass_kernel_spmd

C, H, W = 64, 128, 128
HW = H * W
N_CORES = 8
NPIX = HW              # per-core pixels (per-device BN stats)
BN_EPS = 1e-5
WP = 130               # padded row width (x tiles and z tile)
ZP = 130               # padded side of the conv2 z grid
ZZF = ZP * ZP          # 16900
F16 = mybir.dt.float16
F32 = mybir.dt.float32

NGRP = 8
GR = H // NGRP         # 16 rows per conv/transpose group
NCH = 8                # front-phase chunks (r / relu / kern)
RCH = HW // NCH        # 2048 pixels per chunk
ACT_TABLE_ID = 6       # natural_log_exp_and_others in act_info.json

_CACHE = {}


def _build():
    nc = bacc.Bacc()
    dp = nc.declare_dram_parameter
    x_cm = dp("x_cm", [C, HW], F16, isOutput=False)
    xh0 = dp("xh0", [H, C * WP], F16, isOutput=False)
    wrT = dp("wrT", [C, C], F16, isOutput=False)
    wspanA = dp("wspanA", [C + 1, 9], F16, isOutput=False)
    ones_row = dp("ones_row", [1, HW], F16, isOutput=False)
    ident = dp("ident", [H, 3 * H], F16, isOutput=False)
    w_vert = [dp(f"wv{i}", [2 * C, C], F16, isOutput=False) for i in range(3)]
    w_csing = dp("wcs", [C, 3 * C], F16, isOutput=False)
    gamma = dp("gamma", [C, 1], F32, isOutput=False)
    beta = dp("beta", [C, 1], F32, isOutput=False)
    bconv = dp("bconv", [C, 1], F32, isOutput=False)
    y_out = dp("y", [C, HW], F16, isOutput=True)

    AF = mybir.ActivationFunctionType
    OP = mybir.AluOpType

    with tile.TileContext(nc) as tc:
        with (
            tc.tile_pool(name="sbuf", bufs=1) as pool,
            tc.tile_pool(name="rot", bufs=3) as rot,
            tc.tile_pool(name="yrot", bufs=2) as yrot,
            tc.tile_pool(name="psum", bufs=2, space="PSUM") as pp,
            tc.tile_pool(name="dram", bufs=1, space="DRAM") as dram,
        ):
            # ---- input loads; x_cm first (it gates BN stats) -------------
            # sync/scalar are HWDGE (~0.6us/kick); gpsimd is SWDGE (~1us).
            # Each queue processes its kicks serially, so the critical
            # x_cm + wrT go first on the two HWDGE queues and the bulk xh
            # copies trail them; small/late weights ride gpsimd.
            # tiny critical weights lead each ring; x_cm next, split across
            # all three rings; bulk xh0 trails.
            t_wrT = pool.tile([C, C], F16)
            nc.sync.dma_start(t_wrT[:], wrT[:])
            t_gamma = pool.tile([C, 1], F32)
            t_beta = pool.tile([C, 1], F32)
            t_bconv = pool.tile([C, 1], F32)
            nc.scalar.dma_start(t_gamma[:], gamma[:])
            nc.scalar.dma_start(t_beta[:], beta[:])
            t_wspanA = pool.tile([C + 1, 9], F16)
            nc.gpsimd.dma_start(t_wspanA[:], wspanA[:])
            nc.gpsimd.dma_start(t_bconv[:], bconv[:])
            t_xcm = pool.tile([C, HW], F16)
            qs2 = [nc.sync, nc.scalar]
            XCH = HW // 4
            for q in range(4):
                qs2[q % 2].dma_start(
                    t_xcm[:, q * XCH : (q + 1) * XCH],
                    x_cm[:, q * XCH : (q + 1) * XCH],
                )
            # identity + super/sub-diagonal stationaries for the PE-side
            # involution accumulate (the +-1 h-shift rides the matmul)
            t_idn = [pool.tile([H, H], F16, name=f"idn{i}") for i in range(3)]
            for i in range(3):
                nc.gpsimd.dma_start(t_idn[i][:], ident[:, i * H : (i + 1) * H])
            t_wv = [pool.tile([2 * C, C], F16, name=f"twv{i}") for i in range(3)]
            for i in range(3):
                nc.gpsimd.dma_start(t_wv[i][:], w_vert[i][:])
            t_wc = [pool.tile([C, C], F16, name=f"twc{i}") for i in range(3)]
            for i in range(3):
                nc.gpsimd.dma_start(t_wc[i][:], w_csing[:, i * C : (i + 1) * C])
            t_rraw = pool.tile([C, HW], F16)
            t_rn = pool.tile([C + 1, HW], F16)
            nc.sync.dma_start(t_rn[C : C + 1, :], ones_row[:])
            # bulk xh0 queues behind everything critical (HWDGE rings only —
            # the gpsimd SWDGE ring moves data ~3x slower)
            t_xh0 = pool.tile([H, C * WP], F16)
            nc.sync.dma_start(t_xh0[0 : H // 2, :], xh0[0 : H // 2, :])
            nc.scalar.dma_start(t_xh0[H // 2 : H, :], xh0[H // 2 : H, :])

            # preload the ln/exp table (Copy, Relu, Exp, Ln, Square)
            nc.scalar.add_instruction(
                mybir.InstLoadActFuncSet(
                    name="preload_act_tbl", ins=[], outs=[],
                    act_func_set_id=ACT_TABLE_ID,
                )
            )

            # ---- r = w_reduce @ x, with per-chunk stat accumulation ------
            t_s12 = pool.tile([C, 2 * NCH], F32)
            t_s1a = t_s12[:, 0:NCH]
            t_s2a = t_s12[:, NCH : 2 * NCH]
            for j in range(NCH):
                ps_r = pp.tile([C, RCH], F32, tag="ps")
                for q in range(RCH // 512):
                    o0 = j * RCH + q * 512
                    nc.tensor.matmul(
                        ps_r[:, q * 512 : (q + 1) * 512],
                        lhsT=t_wrT[:],
                        rhs=t_xcm[:, o0 : o0 + 512],
                    )
                rr = t_rraw[:, j * RCH : (j + 1) * RCH]
                nc.scalar.activation(
                    rr, ps_r[:], AF.Copy, accum_out=t_s1a[:, j : j + 1]
                )
                # square scratch lands in t_rn, relu overwrites it later
                nc.vector.scalar_tensor_tensor(
                    out=t_rn[0:C, j * RCH : (j + 1) * RCH], in0=rr, scalar=1.0,
                    in1=rr, op0=OP.mult, op1=OP.mult,
                    accum_out=t_s2a[:, j : j + 1],
                )

            # ---- per-device BN stats -> affine (a, bb) -------------------
            # var = s2/N - m^2; a = gamma/sqrt(var+eps); bb = beta - a*m
            t_s12r = pool.tile([C, 2], F32)
            nc.vector.tensor_reduce(
                t_s12r[:],
                t_s12[:].rearrange("c (s j) -> c s j", s=2),
                axis=mybir.AxisListType.X, op=OP.add,
            )
            t_s1 = t_s12r[:, 0:1]
            t_s2 = t_s12r[:, 1:2]
            t_m = pool.tile([C, 1], F32)
            nc.vector.tensor_scalar_mul(t_m[:], t_s1, 1.0 / NPIX)
            t_m2 = pool.tile([C, 1], F32)
            nc.vector.tensor_tensor(out=t_m2[:], in0=t_m[:], in1=t_m[:], op=OP.mult)
            t_v = pool.tile([C, 1], F32)
            nc.vector.scalar_tensor_tensor(
                out=t_v[:], in0=t_s2, scalar=1.0 / NPIX, in1=t_m2[:],
                op0=OP.mult, op1=OP.subtract,
            )
            t_eps = pool.tile([C, 1], F32)
            nc.vector.memset(t_eps[:], BN_EPS)
            t_lnv = pool.tile([C, 1], F32)
            nc.scalar.activation(t_lnv[:], t_v[:], AF.Ln, bias=t_eps[:])
            t_rstd = pool.tile([C, 1], F32)
            nc.scalar.activation(t_rstd[:], t_lnv[:], AF.Exp, scale=-0.5)
            t_a = pool.tile([C, 1], F32)
            nc.vector.tensor_tensor(out=t_a[:], in0=t_gamma[:], in1=t_rstd[:], op=OP.mult)
            t_ma = pool.tile([C, 1], F32)
            nc.vector.tensor_tensor(out=t_ma[:], in0=t_m[:], in1=t_a[:], op=OP.mult)
            t_bb = pool.tile([C, 1], F32)
            nc.vector.tensor_tensor(out=t_bb[:], in0=t_beta[:], in1=t_ma[:], op=OP.subtract)

            # ---- relu -> kern (c-major) -> drain -> DRAM bounce, 8 chunks
            # t_kcm reuses the x_cm region (x_cm is dead after the r matmul).
            # Three h-major copies come back from the bounce: aligned (t_kern)
            # and row-shifted kernM/kernP (kernM[t] = kern[t+1], kernP[t] =
            # kern[t-1]) so every MAC operand stays partition-aligned; the
            # +-1 output shift is applied by the super/sub-diagonal
            # stationaries in the PE accumulate.
            t_kcm = pool.tile([9, HW], F16, tag="t_xcm")
            t_kern = pool.tile([H, 9 * W], F16)
            t_kernM = pool.tile([H, 9 * W], F16)
            t_kernP = pool.tile([H, 9 * W], F16)
            nc.vector.memset(t_kernM[:], 0.0)
            nc.vector.memset(t_kernP[:], 0.0)
            d_kern = dram.tile([H, 9 * W], F16)
            dkw = d_kern[:].rearrange("h (k w) -> k h w", w=W)
            QS = [nc.sync, nc.gpsimd]
            for j in range(NCH):
                sl = slice(j * RCH, (j + 1) * RCH)
                nc.scalar.activation(
                    t_rn[0:C, sl], t_rraw[:, sl], AF.Relu,
                    bias=t_bb[:], scale=t_a[:],
                )
                ps_k = pp.tile([9, RCH], F32, tag="ps")
                for q in range(RCH // 512):
                    o0 = j * RCH + q * 512
                    nc.tensor.matmul(
                        ps_k[:, q * 512 : (q + 1) * 512],
                        lhsT=t_wspanA[:],
                        rhs=t_rn[:, o0 : o0 + 512],
                    )
                nc.vector.tensor_copy(out=t_kcm[:, sl], in_=ps_k[:])
                # bounce pairs of h-chunks (32 rows) into h-major DRAM
                if j % 2 == 1:
                    h0 = (j - 1) * (H // NCH)
                    nc.sync.dma_start(
                        dkw[:, h0 : h0 + 32, :],
                        t_kcm[:, (j - 1) * RCH : (j + 1) * RCH].rearrange(
                            "k (h w) -> k h w", w=W
                        ),
                    )
                if j == 3:
                    nc.scalar.dma_start(t_kern[0:64, :], d_kern[0:64, :])
                    nc.sync.dma_start(t_kernM[0:63, :], d_kern[1:64, :])
                    nc.scalar.dma_start(t_kernP[1:65, :], d_kern[0:64, :])
                if j == 7:
                    nc.scalar.dma_start(t_kern[64:128, :], d_kern[64:128, :])
                    nc.sync.dma_start(t_kernM[63:127, :], d_kern[64:128, :])
                    nc.scalar.dma_start(t_kernP[65:128, :], d_kern[64:127, :])

            # ---- involution MAC: DVE mults, PE accumulates in PSUM -------
            xv = t_xh0[:].rearrange("h (c w) -> h c w", w=WP)
            ktile = [t_kernM, t_kernM, t_kernM,
                     t_kern, t_kern, t_kern,
                     t_kernP, t_kernP, t_kernP]
            stat = [t_idn[1], t_idn[1], t_idn[1],
                    t_idn[0], t_idn[0], t_idn[0],
                    t_idn[2], t_idn[2], t_idn[2]]

            def x_sl(k, cs):
                j = k % 3
                return xv[:, cs, j : j + W]

            def k_bc(k, n):
                return (
                    ktile[k][:, k * W : (k + 1) * W]
                    .rearrange("h (o w) -> h o w", o=1)
                    .broadcast_to([H, n, W])
                )

            # z tile reuses the rraw region (dead after relu)
            t_z = pool.tile([H, C * WP], F16, tag="t_rraw")
            zvp = t_z[:].rearrange("h (c w) -> h c w", w=WP)
            nc.vector.memset(zvp[:, :, 0], 0.0)
            nc.vector.memset(zvp[:, :, WP - 1], 0.0)

            d_z = dram.tile([C, ZZF], F16)
            dzw = d_z[:].rearrange("c (a b) -> a c b", b=ZP)
            t_zrow = pool.tile([C, ZP], F16)
            nc.vector.memset(t_zrow[:], 0.0)
            nc.gpsimd.dma_start(dzw[0, :, :], t_zrow[:])
            nc.gpsimd.dma_start(dzw[ZP - 1, :, :], t_zrow[:])

            # conv2 stacked tile: T1 = [zpad ; zpad+260] reuses the
            # x_cm/kern_cm region (vertical tap pairs contract 128-deep;
            # center-row taps read the upper half at row offset +1).
            t_T1 = pool.tile([2 * C, ZZF], F16, tag="t_xcm")

            def tz_legs(cc):
                """Direct DRAM read legs for one c-quarter: upper half is the
                plain zpad, lower half reads the same rows shifted +260 (conv
                reads shifted-flat indices up to 16639)."""
                ch0 = cc * (C // 4)
                CH = C // 4
                chs = slice(ch0, ch0 + CH)
                LEN = ZZF - 260
                nc.sync.dma_start(t_T1[ch0 : ch0 + CH, :], d_z[chs, :])
                nc.scalar.dma_start(
                    t_T1[C + ch0 : C + ch0 + CH, 0:LEN], d_z[chs, 260:ZZF]
                )

            CCH = C // 4
            for cc in range(4):
                c0 = cc * CCH
                cs = slice(c0, c0 + CCH)
                ps_acc = pp.tile([H, RCH], F32, tag="ps")
                for k in range(9):
                    t_tmp = rot.tile([H, CCH * W], F16, tag="tmp")
                    tv = t_tmp[:].rearrange("h (c w) -> h c w", w=W)
                    nc.vector.tensor_tensor(
                        out=tv, in0=x_sl(k, cs), in1=k_bc(k, CCH), op=OP.mult
                    )
                    for q in range(4):
                        nc.tensor.matmul(
                            ps_acc[:, q * 512 : (q + 1) * 512],
                            lhsT=stat[k][:],
                            rhs=t_tmp[:, q * 512 : (q + 1) * 512],
                            start=(k == 0),
                            stop=(k == 8),
                        )
                # softplus + write-leg per c-sub-chunk (shorter tail chain)
                for hf in range(2):
                    csub = slice(c0 + hf * 8, c0 + (hf + 1) * 8)
                    t_esp = rot.tile([H, 8 * W], F16, tag="esp")
                    nc.scalar.activation(
                        t_esp[:], ps_acc[:, hf * 1024 : (hf + 1) * 1024], AF.Exp
                    )
                    nc.scalar.activation(
                        zvp[:, csub, 1 : 1 + W],
                        t_esp[:].rearrange("h (c w) -> h c w", w=W),
                        AF.Ln, bias=1.0,
                    )
                    nc.sync.dma_start(dzw[1 : 1 + H, csub, :], zvp[:, csub, :])
                tz_legs(cc)

            # ---- conv2: 6 matmul passes per 16-row group -----------------
            T1v = t_T1[:].rearrange("p (a b) -> p a b", b=ZP)

            def conv_grp(g):
                # pass-major: 4 consecutive matmuls share each stationary
                ps_y = pp.tile([C, GR * W], F32, tag="ps")
                for p in range(6):
                    kw = p % 3
                    for sub in range(GR // 4):
                        r0 = g * GR + sub * 4
                        o = ps_y[:, sub * 512 : (sub + 1) * 512]
                        if p < 3:
                            nc.tensor.matmul(
                                o, lhsT=t_wv[kw][:],
                                rhs=T1v[:, r0 : r0 + 4, kw : kw + W],
                                start=(p == 0), stop=False,
                            )
                        else:
                            nc.tensor.matmul(
                                o, lhsT=t_wc[kw][:],
                                rhs=T1v[0:C, r0 + 1 : r0 + 5, kw : kw + W],
                                start=False, stop=(p == 5),
                            )
                t_ey = yrot.tile([C, GR * W], F16, tag="ey")
                nc.scalar.activation(t_ey[:], ps_y[:], AF.Exp, bias=t_bconv[:])
                t_y = yrot.tile([C, GR * W], F16, tag="yc")
                nc.scalar.activation(t_y[:], t_ey[:], AF.Ln, bias=1.0)
                (nc.sync if g % 2 == 0 else nc.scalar).dma_start(
                    y_out[:, g * GR * W : (g + 1) * GR * W], t_y[:]
                )

            for g in range(NGRP):
                conv_grp(g)

    nc.compile()
    return nc


def _prep_core_inputs(xs, w_reduce, b_reduce, bn_gamma, bn_beta, w_span, b_span,
                      w_conv2, b_conv2):
    """Host-side layout prep for one core's sample xs [C, H, W] fp32."""
    xhw = xs.transpose(1, 0, 2).astype(np.float16)  # [h, c, w]
    xh0 = np.zeros((H, C, WP), np.float16)
    xh0[:, :, 1 : 1 + W] = xhw
    idn = np.concatenate(
        [np.eye(H, dtype=np.float16),
         np.eye(H, k=1, dtype=np.float16),
         np.eye(H, k=-1, dtype=np.float16)], axis=1
    )
    m = {
        "x_cm": xs.reshape(C, HW).astype(np.float16),
        "xh0": xh0.reshape(H, C * WP),
        "wrT": np.ascontiguousarray(w_reduce.T).astype(np.float16),
        "wspanA": np.ascontiguousarray(
            np.vstack([w_span.T, b_span[None, :]])
        ).astype(np.float16),
        "ones_row": np.ones((1, HW), np.float16),
        "ident": np.ascontiguousarray(idn),
        "gamma": bn_gamma.astype(np.float32).reshape(C, 1),
        "beta": bn_beta.astype(np.float32).reshape(C, 1),
        "bconv": b_conv2.astype(np.float32).reshape(C, 1),
    }
    for kw in range(3):
        wv = np.concatenate(
            [w_conv2[:, :, 0, kw].T, w_conv2[:, :, 2, kw].T], axis=0
        ).astype(np.float16)
        m[f"wv{kw}"] = np.ascontiguousarray(wv)
    m["wcs"] = np.ascontiguousarray(
        np.concatenate([w_conv2[:, :, 1, kw].T for kw in range(3)], axis=1)
    ).astype(np.float16)
    return m


def kernel(x, w_reduce, b_reduce, bn_gamma, bn_beta, w_span, b_span, w_conv2,
           b_conv2):
    x = np.asarray(x, np.float32)
    if "nc" not in _CACHE:
        _CACHE["nc"] = _build()
    nc = _CACHE["nc"]
    in_maps = [
        _prep_core_inputs(
            x[b], np.asarray(w_reduce, np.float32), np.asarray(b_reduce, np.float32),
            np.asarray(bn_gamma, np.float32), np.asarray(bn_beta, np.float32),
            np.asarray(w_span, np.float32), np.asarray(b_span, np.float32),
            np.asarray(w_conv2, np.float32), np.asarray(b_conv2, np.float32),
        )
        for b in range(N_CORES)
    ]
    res = run_bass_kernel_spmd(nc, in_maps, core_ids=list(range(N_CORES)))
    out = np.stack([res.results[b]["y"].reshape(C, H, W) for b in range(N_CORES)])
    return out.astype(np.float32)


# revision 42
# speedup vs baseline: 1.0555x; 1.0555x over previous
"""Trainium2 Bass kernel for nn_Block_en_49469433315543 (involution block).

Computation (see reference):
  z = softplus(involution(x))          involution: per-pixel 3x3 dynamic kernel
  y = softplus(conv2d_3x3(z) + b_conv2)
with the per-pixel kernel = w_span @ relu(BN(w_reduce @ x)) + b_span.

Sharding: data-parallel over batch, one sample per NeuronCore (8 cores).
BN uses per-device batch statistics (sanctioned by the sharding spec);
measured deviation ~4e-3 vs the 2e-2 gate.  No collectives.

Schedule (v2, from trace analysis of the 219us baseline):
  - Involution MAC: DVE computes only the 9 per-tap products tmp_k =
    kern_k (x) x_shift_k; the TENSOR engine accumulates them into PSUM via
    identity matmuls (psum += I^T @ tmp_k, exact in fp32).  This halves the
    DVE-bound MAC phase (measured DVE 2-stream fp16 rate ~0.6 ns/elem; the
    8 add passes move to the otherwise-idle PE).
  - kern = wspanA^T @ [rn;1] computed c-major in 8 h-chunks (one stationary
    weight, 4x512-col matmuls per chunk) -> DVE drain -> DRAM bounce into
    [H, 9, W] h-major (144 x 256B descriptors per chunk).
  - Front pipeline: x_cm lands first; r-matmul chunks chase the load; BN
    stats at ~10us; relu/kern/drain/bounce in 8 pipelined h-chunks.
  - conv2 in 5 matmul passes per row group (vertical tap pairs (kh=0,kh=2)
    contract 128-deep via the stacked T1 = [zpad; zpad+260] tile; center
    row pair via T2 = [zpad+130; zpad+132]; one 64-deep single).
  - z transpose bounces via DRAM per (wave, row-group); T1-lower/T2 are
    flat shifted SBUF->SBUF copies of the landed zpad.
  - GpSimd never computes (measured 4x DVE slowdown from SBUF contention);
    it only kicks software-DGE DMAs.
"""
import sys

for _p in ("/opt/trn_rl_repo", "/root/.axon_site/_ro/trn_rl_repo"):
    if _p not in sys.path:
        sys.path.insert(0, _p)

import numpy as np

import concourse.bacc as bacc
import concourse.tile as tile
from concourse import mybir
from concourse.bass_utils import run_bass_kernel_spmd

C, H, W = 64, 128, 128
HW = H * W
N_CORES = 8
NPIX = HW              # per-core pixels (per-device BN stats)
BN_EPS = 1e-5
WP = 130               # padded row width (x tiles and z tile)
ZP = 130               # padded side of the conv2 z grid
ZZF = ZP * ZP          # 16900
F16 = mybir.dt.float16
F32 = mybir.dt.float32

NGRP = 8
GR = H // NGRP         # 16 rows per conv/transpose group
NCH = 8                # front-phase chunks (r / relu / kern)
RCH = HW // NCH        # 2048 pixels per chunk
ACT_TABLE_ID = 6       # natural_log_exp_and_others in act_info.json

_CACHE = {}


def _build():
    nc = bacc.Bacc()
    dp = nc.declare_dram_parameter
    x_cm = dp("x_cm", [C, HW], F16, isOutput=False)
    xh0 = dp("xh0", [H, C * WP], F16, isOutput=False)
    wrT = dp("wrT", [C, C], F16, isOutput=False)
    wspanA = dp("wspanA", [C + 1, 9], F16, isOutput=False)
    ones_row = dp("ones_row", [1, HW], F16, isOutput=False)
    ident = dp("ident", [H, 3 * H], F16, isOutput=False)
    w_vert = [dp(f"wv{i}", [2 * C, C], F16, isOutput=False) for i in range(3)]
    w_csing = dp("wcs", [C, 3 * C], F16, isOutput=False)
    gamma = dp("gamma", [C, 1], F32, isOutput=False)
    beta = dp("beta", [C, 1], F32, isOutput=False)
    bconv = dp("bconv", [C, 1], F32, isOutput=False)
    y_out = dp("y", [C, HW], F16, isOutput=True)

    AF = mybir.ActivationFunctionType
    OP = mybir.AluOpType

    with tile.TileContext(nc) as tc:
        with (
            tc.tile_pool(name="sbuf", bufs=1) as pool,
            tc.tile_pool(name="rot", bufs=3) as rot,
            tc.tile_pool(name="yrot", bufs=2) as yrot,
            tc.tile_pool(name="psum", bufs=2, space="PSUM") as pp,
            tc.tile_pool(name="dram", bufs=1, space="DRAM") as dram,
        ):
            # ---- input loads; x_cm first (it gates BN stats) -------------
            # sync/scalar are HWDGE (~0.6us/kick); gpsimd is SWDGE (~1us).
            # Each queue processes its kicks serially, so the critical
            # x_cm + wrT go first on the two HWDGE queues and the bulk xh
            # copies trail them; small/late weights ride gpsimd.
            # tiny critical weights lead each ring; x_cm next, split across
            # all three rings; bulk xh0 trails.
            t_wrT = pool.tile([C, C], F16)
            nc.sync.dma_start(t_wrT[:], wrT[:])
            t_gamma = pool.tile([C, 1], F32)
            t_beta = pool.tile([C, 1], F32)
            t_bconv = pool.tile([C, 1], F32)
            nc.scalar.dma_start(t_gamma[:], gamma[:])
            nc.scalar.dma_start(t_beta[:], beta[:])
            t_wspanA = pool.tile([C + 1, 9], F16)
            nc.gpsimd.dma_start(t_wspanA[:], wspanA[:])
            nc.gpsimd.dma_start(t_bconv[:], bconv[:])
            t_xcm = pool.tile([C, HW], F16)
            qs2 = [nc.sync, nc.scalar]
            XCH = HW // 4
            for q in range(4):
                qs2[q % 2].dma_start(
                    t_xcm[:, q * XCH : (q + 1) * XCH],
                    x_cm[:, q * XCH : (q + 1) * XCH],
                )
            # identity + super/sub-diagonal stationaries for the PE-side
            # involution accumulate (the +-1 h-shift rides the matmul)
            t_idn = [pool.tile([H, H], F16, name=f"idn{i}") for i in range(3)]
            for i in range(3):
                nc.gpsimd.dma_start(t_idn[i][:], ident[:, i * H : (i + 1) * H])
            t_wv = [pool.tile([2 * C, C], F16, name=f"twv{i}") for i in range(3)]
            for i in range(3):
                nc.gpsimd.dma_start(t_wv[i][:], w_vert[i][:])
            t_wc = [pool.tile([C, C], F16, name=f"twc{i}") for i in range(3)]
            for i in range(3):
                nc.gpsimd.dma_start(t_wc[i][:], w_csing[:, i * C : (i + 1) * C])
            t_rraw = pool.tile([C, HW], F16)
            t_rn = pool.tile([C + 1, HW], F16)
            nc.sync.dma_start(t_rn[C : C + 1, :], ones_row[:])
            # bulk xh0 queues behind everything critical (HWDGE rings only —
            # the gpsimd SWDGE ring moves data ~3x slower)
            t_xh0 = pool.tile([H, C * WP], F16)
            nc.sync.dma_start(t_xh0[0 : H // 2, :], xh0[0 : H // 2, :])
            nc.scalar.dma_start(t_xh0[H // 2 : H, :], xh0[H // 2 : H, :])

            # preload the ln/exp table (Copy, Relu, Exp, Ln, Square)
            nc.scalar.add_instruction(
                mybir.InstLoadActFuncSet(
                    name="preload_act_tbl", ins=[], outs=[],
                    act_func_set_id=ACT_TABLE_ID,
                )
            )

            # ---- r = w_reduce @ x, with per-chunk stat accumulation ------
            t_s12 = pool.tile([C, 2 * NCH], F32)
            t_s1a = t_s12[:, 0:NCH]
            t_s2a = t_s12[:, NCH : 2 * NCH]
            for j in range(NCH):
                ps_r = pp.tile([C, RCH], F32, tag="ps")
                for q in range(RCH // 512):
                    o0 = j * RCH + q * 512
                    nc.tensor.matmul(
                        ps_r[:, q * 512 : (q + 1) * 512],
                        lhsT=t_wrT[:],
                        rhs=t_xcm[:, o0 : o0 + 512],
                    )
                rr = t_rraw[:, j * RCH : (j + 1) * RCH]
                nc.scalar.activation(
                    rr, ps_r[:], AF.Copy, accum_out=t_s1a[:, j : j + 1]
                )
                # square scratch lands in t_rn, relu overwrites it later
                nc.vector.scalar_tensor_tensor(
                    out=t_rn[0:C, j * RCH : (j + 1) * RCH], in0=rr, scalar=1.0,
                    in1=rr, op0=OP.mult, op1=OP.mult,
                    accum_out=t_s2a[:, j : j + 1],
                )

            # ---- per-device BN stats -> affine (a, bb) -------------------
            # var = s2/N - m^2; a = gamma/sqrt(var+eps); bb = beta - a*m
            t_s12r = pool.tile([C, 2], F32)
            nc.vector.tensor_reduce(
                t_s12r[:],
                t_s12[:].rearrange("c (s j) -> c s j", s=2),
                axis=mybir.AxisListType.X, op=OP.add,
            )
            t_s1 = t_s12r[:, 0:1]
            t_s2 = t_s12r[:, 1:2]
            t_m = pool.tile([C, 1], F32)
            nc.vector.tensor_scalar_mul(t_m[:], t_s1, 1.0 / NPIX)
            t_m2 = pool.tile([C, 1], F32)
            nc.vector.tensor_tensor(out=t_m2[:], in0=t_m[:], in1=t_m[:], op=OP.mult)
            t_v = pool.tile([C, 1], F32)
            nc.vector.scalar_tensor_tensor(
                out=t_v[:], in0=t_s2, scalar=1.0 / NPIX, in1=t_m2[:],
                op0=OP.mult, op1=OP.subtract,
            )
            t_eps = pool.tile([C, 1], F32)
            nc.vector.memset(t_eps[:], BN_EPS)
            t_lnv = pool.tile([C, 1], F32)
            nc.scalar.activation(t_lnv[:], t_v[:], AF.Ln, bias=t_eps[:])
            t_rstd = pool.tile([C, 1], F32)
            nc.scalar.activation(t_rstd[:], t_lnv[:], AF.Exp, scale=-0.5)
            t_a = pool.tile([C, 1], F32)
            nc.vector.tensor_tensor(out=t_a[:], in0=t_gamma[:], in1=t_rstd[:], op=OP.mult)
            t_ma = pool.tile([C, 1], F32)
            nc.vector.tensor_tensor(out=t_ma[:], in0=t_m[:], in1=t_a[:], op=OP.mult)
            t_bb = pool.tile([C, 1], F32)
            nc.vector.tensor_tensor(out=t_bb[:], in0=t_beta[:], in1=t_ma[:], op=OP.subtract)

            # ---- relu -> kern (c-major) -> drain -> DRAM bounce, 8 chunks
            # t_kcm reuses the x_cm region (x_cm is dead after the r matmul).
            # Three h-major copies come back from the bounce: aligned (t_kern)
            # and row-shifted kernM/kernP (kernM[t] = kern[t+1], kernP[t] =
            # kern[t-1]) so every MAC operand stays partition-aligned; the
            # +-1 output shift is applied by the super/sub-diagonal
            # stationaries in the PE accumulate.
            t_kcm = pool.tile([9, HW], F16, tag="t_xcm")
            t_kern = pool.tile([H, 9 * W], F16)
            t_kernM = pool.tile([H, 9 * W], F16)
            t_kernP = pool.tile([H, 9 * W], F16)
            nc.vector.memset(t_kernM[:], 0.0)
            nc.vector.memset(t_kernP[:], 0.0)
            d_kern = dram.tile([H, 9 * W], F16)
            dkw = d_kern[:].rearrange("h (k w) -> k h w", w=W)
            QS = [nc.sync, nc.gpsimd]
            for j in range(NCH):
                sl = slice(j * RCH, (j + 1) * RCH)
                nc.scalar.activation(
                    t_rn[0:C, sl], t_rraw[:, sl], AF.Relu,
                    bias=t_bb[:], scale=t_a[:],
                )
                ps_k = pp.tile([9, RCH], F32, tag="ps")
                for q in range(RCH // 512):
                    o0 = j * RCH + q * 512
                    nc.tensor.matmul(
                        ps_k[:, q * 512 : (q + 1) * 512],
                        lhsT=t_wspanA[:],
                        rhs=t_rn[:, o0 : o0 + 512],
                    )
                nc.vector.tensor_copy(out=t_kcm[:, sl], in_=ps_k[:])
                # bounce pairs of h-chunks (32 rows) into h-major DRAM
                if j % 2 == 1:
                    h0 = (j - 1) * (H // NCH)
                    nc.sync.dma_start(
                        dkw[:, h0 : h0 + 32, :],
                        t_kcm[:, (j - 1) * RCH : (j + 1) * RCH].rearrange(
                            "k (h w) -> k h w", w=W
                        ),
                    )
                if j == 3:
                    nc.scalar.dma_start(t_kern[0:64, :], d_kern[0:64, :])
                    nc.sync.dma_start(t_kernM[0:63, :], d_kern[1:64, :])
                    nc.scalar.dma_start(t_kernP[1:65, :], d_kern[0:64, :])
                if j == 7:
                    nc.scalar.dma_start(t_kern[64:128, :], d_kern[64:128, :])
                    nc.sync.dma_start(t_kernM[63:127, :], d_kern[64:128, :])
                    nc.scalar.dma_start(t_kernP[65:128, :], d_kern[64:127, :])

            # ---- involution MAC: DVE mults, PE accumulates in PSUM -------
            xv = t_xh0[:].rearrange("h (c w) -> h c w", w=WP)
            ktile = [t_kernM, t_kernM, t_kernM,
                     t_kern, t_kern, t_kern,
                     t_kernP, t_kernP, t_kernP]
            stat = [t_idn[1], t_idn[1], t_idn[1],
                    t_idn[0], t_idn[0], t_idn[0],
                    t_idn[2], t_idn[2], t_idn[2]]

            def x_sl(k, cs):
                j = k % 3
                return xv[:, cs, j : j + W]

            def k_bc(k, n):
                return (
                    ktile[k][:, k * W : (k + 1) * W]
                    .rearrange("h (o w) -> h o w", o=1)
                    .broadcast_to([H, n, W])
                )

            # z tile reuses the rraw region (dead after relu)
            t_z = pool.tile([H, C * WP], F16, tag="t_rraw")
            zvp = t_z[:].rearrange("h (c w) -> h c w", w=WP)
            nc.vector.memset(zvp[:, :, 0], 0.0)
            nc.vector.memset(zvp[:, :, WP - 1], 0.0)

            d_z = dram.tile([C, ZZF], F16)
            dzw = d_z[:].rearrange("c (a b) -> a c b", b=ZP)
            t_zrow = pool.tile([C, ZP], F16)
            nc.vector.memset(t_zrow[:], 0.0)
            nc.gpsimd.dma_start(dzw[0, :, :], t_zrow[:])
            nc.gpsimd.dma_start(dzw[ZP - 1, :, :], t_zrow[:])

            # conv2 stacked tile: T1 = [zpad ; zpad+260] reuses the
            # x_cm/kern_cm region (vertical tap pairs contract 128-deep;
            # center-row taps read the upper half at row offset +1).
            t_T1 = pool.tile([2 * C, ZZF], F16, tag="t_xcm")

            def tz_legs(half):
                """Direct DRAM read legs for one c-half: upper half is the
                plain zpad, lower half reads the same rows shifted +260 (conv
                reads shifted-flat indices up to 16639)."""
                ch0 = half * (C // 2)
                CH = C // 2
                chs = slice(ch0, ch0 + CH)
                LEN = ZZF - 260
                nc.sync.dma_start(t_T1[ch0 : ch0 + CH, :], d_z[chs, :])
                nc.scalar.dma_start(
                    t_T1[C + ch0 : C + ch0 + CH, 0:LEN], d_z[chs, 260:ZZF]
                )

            CCH = C // 4
            for cc in range(4):
                c0 = cc * CCH
                cs = slice(c0, c0 + CCH)
                ps_acc = pp.tile([H, RCH], F32, tag="ps")
                for k in range(9):
                    t_tmp = rot.tile([H, CCH * W], F16, tag="tmp")
                    tv = t_tmp[:].rearrange("h (c w) -> h c w", w=W)
                    nc.vector.tensor_tensor(
                        out=tv, in0=x_sl(k, cs), in1=k_bc(k, CCH), op=OP.mult
                    )
                    for q in range(4):
                        nc.tensor.matmul(
                            ps_acc[:, q * 512 : (q + 1) * 512],
                            lhsT=stat[k][:],
                            rhs=t_tmp[:, q * 512 : (q + 1) * 512],
                            start=(k == 0),
                            stop=(k == 8),
                        )
                # softplus + write-leg per c-sub-chunk (shorter tail chain)
                for hf in range(2):
                    csub = slice(c0 + hf * 8, c0 + (hf + 1) * 8)
                    t_esp = rot.tile([H, 8 * W], F16, tag="esp")
                    nc.scalar.activation(
                        t_esp[:], ps_acc[:, hf * 1024 : (hf + 1) * 1024], AF.Exp
                    )
                    nc.scalar.activation(
                        zvp[:, csub, 1 : 1 + W],
                        t_esp[:].rearrange("h (c w) -> h c w", w=W),
                        AF.Ln, bias=1.0,
                    )
                    nc.sync.dma_start(dzw[1 : 1 + H, csub, :], zvp[:, csub, :])
                if cc in (1, 3):
                    tz_legs(cc // 2)

            # ---- conv2: 6 matmul passes per 16-row group -----------------
            T1v = t_T1[:].rearrange("p (a b) -> p a b", b=ZP)

            def conv_grp(g):
                # pass-major: 4 consecutive matmuls share each stationary
                ps_y = pp.tile([C, GR * W], F32, tag="ps")
                for p in range(6):
                    kw = p % 3
                    for sub in range(GR // 4):
                        r0 = g * GR + sub * 4
                        o = ps_y[:, sub * 512 : (sub + 1) * 512]
                        if p < 3:
                            nc.tensor.matmul(
                                o, lhsT=t_wv[kw][:],
                                rhs=T1v[:, r0 : r0 + 4, kw : kw + W],
                                start=(p == 0), stop=False,
                            )
                        else:
                            nc.tensor.matmul(
                                o, lhsT=t_wc[kw][:],
                                rhs=T1v[0:C, r0 + 1 : r0 + 5, kw : kw + W],
                                start=False, stop=(p == 5),
                            )
                t_ey = yrot.tile([C, GR * W], F16, tag="ey")
                nc.scalar.activation(t_ey[:], ps_y[:], AF.Exp, bias=t_bconv[:])
                t_y = yrot.tile([C, GR * W], F16, tag="yc")
                nc.scalar.activation(t_y[:], t_ey[:], AF.Ln, bias=1.0)
                (nc.sync if g % 2 == 0 else nc.scalar).dma_start(
                    y_out[:, g * GR * W : (g + 1) * GR * W], t_y[:]
                )

            for g in range(NGRP):
                conv_grp(g)

    nc.compile()
    return nc


def _prep_core_inputs(xs, w_reduce, b_reduce, bn_gamma, bn_beta, w_span, b_span,
                      w_conv2, b_conv2):
    """Host-side layout prep for one core's sample xs [C, H, W] fp32."""
    xhw = xs.transpose(1, 0, 2).astype(np.float16)  # [h, c, w]
    xh0 = np.zeros((H, C, WP), np.float16)
    xh0[:, :, 1 : 1 + W] = xhw
    idn = np.concatenate(
        [np.eye(H, dtype=np.float16),
         np.eye(H, k=1, dtype=np.float16),
         np.eye(H, k=-1, dtype=np.float16)], axis=1
    )
    m = {
        "x_cm": xs.reshape(C, HW).astype(np.float16),
        "xh0": xh0.reshape(H, C * WP),
        "wrT": np.ascontiguousarray(w_reduce.T).astype(np.float16),
        "wspanA": np.ascontiguousarray(
            np.vstack([w_span.T, b_span[None, :]])
        ).astype(np.float16),
        "ones_row": np.ones((1, HW), np.float16),
        "ident": np.ascontiguousarray(idn),
        "gamma": bn_gamma.astype(np.float32).reshape(C, 1),
        "beta": bn_beta.astype(np.float32).reshape(C, 1),
        "bconv": b_conv2.astype(np.float32).reshape(C, 1),
    }
    for kw in range(3):
        wv = np.concatenate(
            [w_conv2[:, :, 0, kw].T, w_conv2[:, :, 2, kw].T], axis=0
        ).astype(np.float16)
        m[f"wv{kw}"] = np.ascontiguousarray(wv)
    m["wcs"] = np.ascontiguousarray(
        np.concatenate([w_conv2[:, :, 1, kw].T for kw in range(3)], axis=1)
    ).astype(np.float16)
    return m


def kernel(x, w_reduce, b_reduce, bn_gamma, bn_beta, w_span, b_span, w_conv2,
           b_conv2):
    x = np.asarray(x, np.float32)
    if "nc" not in _CACHE:
        _CACHE["nc"] = _build()
    nc = _CACHE["nc"]
    in_maps = [
        _prep_core_inputs(
            x[b], np.asarray(w_reduce, np.float32), np.asarray(b_reduce, np.float32),
            np.asarray(bn_gamma, np.float32), np.asarray(bn_beta, np.float32),
            np.asarray(w_span, np.float32), np.asarray(b_span, np.float32),
            np.asarray(w_conv2, np.float32), np.asarray(b_conv2, np.float32),
        )
        for b in range(N_CORES)
    ]
    res = run_bass_kernel_spmd(nc, in_maps, core_ids=list(range(N_CORES)))
    out = np.stack([res.results[b]["y"].reshape(C, H, W) for b in range(N_CORES)])
    return out.astype(np.float32)
